# revision 14
# baseline (speedup 1.0000x reference)
"""Bass/Trainium2 kernel for nn_ClassQueryAttention.

Math (per batch b, x flattened to [C=256, N=16384]):
  logits[k,n] = (qe @ Wk) @ x / sqrt(D)          (per-k bias qe@bk cancels in softmax)
  p = exp(logits)  (no max-subtraction needed: logits ~ N(0,1))
  s_k = sum_n p[k,n];  r_k = 1/s_k
  y[k,c] = sum_n p[k,n] x[c,n]                   (flash-style, accumulated in PSUM)
  xa[c]  = sum_k r_k y[k,c]
  gate   = (Wo@Wv) @ xa + K*(Wo@bv + bo)
  out[c,n] = x[c,n] * gate[c]

Sharding: data-parallel over batch B=8, one batch per NeuronCore, no collectives.
Per-core HBM traffic: 1 read of x (16 MiB) + 1 write (16 MiB) = 32 MiB (the floor).

Layout: the class dim (K=21, padded to 32) is packed 4-wide across PE column
groups - quad g (512 px) of each 2048-px big tile lands on PSUM partitions
32g..32g+31. One full-lane exp per big tile; pT transposes are full [128,128];
everything between the logits matmul and the y matmul is bf16 (tolerance is
2e-2, bf16 keeps us ~1e-3). x is cast to bf16 on the otherwise-idle GpSimd.
"""

import sys
from contextlib import ExitStack

import numpy as np

sys.path.insert(0, "/opt/trn_rl_repo")

import concourse.bass as bass  # noqa: E402
import concourse.tile as tile  # noqa: E402
from concourse import bacc, mybir  # noqa: E402
from concourse.bass_utils import run_bass_kernel_spmd  # noqa: E402

B, C, HW = 8, 256, 128 * 128
K, D = 21, 256
KP = 32          # class dim padded to a full PE column group
P = 128          # partition count / channel chunk
NB = 2048        # DMA big-tile pixels
NQ = 512         # logits quad pixels (one PE column group each)
NS = 128         # transpose subtile pixels
F32 = mybir.dt.float32
BF16 = mybir.dt.bfloat16
AF = mybir.ActivationFunctionType


def _body(ctx: ExitStack, tc: tile.TileContext, x, qk16, m2, g0, identf, ident16,
          sel4, sel4n, out, sfx=""):
    nc = tc.nc

    def pool(name, **kw):
        return ctx.enter_context(tc.tile_pool(name=name + sfx, **kw))

    consts = pool("consts", bufs=1)
    qk0 = consts.tile([P, KP], BF16, tag="qk0")
    qk1 = consts.tile([P, KP], BF16, tag="qk1")
    m2t0 = consts.tile([P, C], F32, tag="m2t0")
    m2t1 = consts.tile([P, C], F32, tag="m2t1")
    g0_sb = consts.tile([P, 2], F32, tag="g0")
    idf_sb = consts.tile([P, P], F32, tag="identf")
    id16 = consts.tile([P, P], BF16, tag="id16")
    selt_sb = consts.tile([K, P], F32, tag="selt")
    sel4n_sb = consts.tile([P, K], F32, tag="sel4n")
    s_acc = consts.tile([P, HW // NB], F32, tag="s_acc")

    nc.sync.dma_start(qk0[:], qk16[0:P, :])
    nc.sync.dma_start(qk1[:], qk16[P : 2 * P, :])
    nc.sync.dma_start(m2t0[:], m2[0:P, :])
    nc.sync.dma_start(m2t1[:], m2[P : 2 * P, :])
    nc.sync.dma_start(g0_sb[:], g0[:, :])
    nc.sync.dma_start(idf_sb[:], identf[:, :])
    nc.sync.dma_start(id16[:], ident16[:, :])
    nc.sync.dma_start(selt_sb[:], sel4[:, :])
    nc.sync.dma_start(sel4n_sb[:], sel4n[:, :])

    xbig = pool("xbig", bufs=1)
    x16p = pool("x16", bufs=3)
    ps_l = pool("ps_l", bufs=2, space="PSUM")      # logits [128,512] f32: 2 banks
    ps_pt = pool("ps_pt", bufs=2, space="PSUM")    # pT [128,4,128] bf16
    ps_xt = pool("ps_xt", bufs=3, space="PSUM")    # xT pair [128,2,256] bf16
    ps_y = pool("ps_y", bufs=1, space="PSUM")      # y accum [128,256] f32
    sb_xt = pool("sb_xt", bufs=4)
    sb_pt = pool("sb_pt", bufs=3)
    sb_p = pool("sb_p", bufs=3)

    # ---------------- Phase A ------------------------------------------------
    # x stays resident in SBUF (f32, for phase C); a rotating bf16 copy feeds
    # the matmuls. Logits are 4x col-group packed (quad g -> partitions 32g+k),
    # so exp runs all 128 lanes and pT transposes are full 128x128.
    y_ps = ps_y.tile([P, C], F32, tag="y")
    n_big = HW // NB                 # 8
    xres = {}
    copy_rr = 0
    for bt in range(n_big):
        xb0 = xbig.tile([P, NB], F32, tag=f"xb0_{bt}")
        xb1 = xbig.tile([P, NB], F32, tag=f"xb1_{bt}")
        xres[0, bt], xres[1, bt] = xb0, xb1
        nc.sync.dma_start(xb0[:], x[0:P, bt * NB : (bt + 1) * NB])
        nc.sync.dma_start(xb1[:], x[P : 2 * P, bt * NB : (bt + 1) * NB])
        x16_0 = x16p.tile([P, NB], BF16, tag="x16_0")
        x16_1 = x16p.tile([P, NB], BF16, tag="x16_1")
        nc.vector.tensor_copy(x16_0[:], xb0[:])
        nc.scalar.copy(x16_1[:], xb1[:])

        # logits for all 4 quads concurrently (one PE col group per quad)
        l_ps = ps_l.tile([P, NQ], F32, tag="l")
        for g in range(4):
            gsl = slice(g * NQ, (g + 1) * NQ)
            nc.tensor.matmul(
                l_ps[32 * g : 32 * (g + 1), :], qk0[:], x16_0[:, gsl],
                start=True, stop=False, skip_group_check=True,
                tile_position=(0, 32 * g),
            )
            nc.tensor.matmul(
                l_ps[32 * g : 32 * (g + 1), :], qk1[:], x16_1[:, gsl],
                start=False, stop=True, skip_group_check=True,
                tile_position=(0, 32 * g),
            )
        p_sb = sb_p.tile([P, NQ], BF16, tag="p")
        nc.scalar.activation(
            p_sb[:], l_ps[:], AF.Exp, accum_out=s_acc[:, bt : bt + 1]
        )
        pt_ps = ps_pt.tile([P, 4, P], BF16, tag="pt")
        for j in range(4):
            nc.tensor.transpose(
                pt_ps[:, j, :], p_sb[:, j * NS : (j + 1) * NS], id16[:, :]
            )
        pt_sb = sb_pt.tile([P, 4, P], BF16, tag="ptsb")
        nc.vector.tensor_copy(pt_sb[:], pt_ps[:])

        # x transposes + y matmuls; chunk ch = 4g + j covers pixels 128*ch..
        for j in range(4):
            for gg in range(2):
                xt_ps = ps_xt.tile([P, 2, C], BF16, tag="xt")
                for gi in range(2):
                    g = 2 * gg + gi
                    ss = slice((4 * g + j) * NS, (4 * g + j + 1) * NS)
                    nc.tensor.transpose(xt_ps[:, gi, 0:P], x16_0[:, ss], id16[:, :])
                    nc.tensor.transpose(xt_ps[:, gi, P : 2 * P], x16_1[:, ss], id16[:, :])
                xt_sb = sb_xt.tile([P, 2, C], BF16, tag="xtsb")
                if copy_rr % 2 == 0:   # 50/50 DVE / ACT
                    nc.vector.tensor_copy(xt_sb[:], xt_ps[:])
                else:
                    nc.scalar.copy(xt_sb[:], xt_ps[:])
                copy_rr += 1
                for gi in range(2):
                    g = 2 * gg + gi
                    nc.tensor.matmul(
                        y_ps[32 * g : 32 * g + K, :],
                        pt_sb[:, j, 32 * g : 32 * g + K],
                        xt_sb[:, gi, :],
                        start=(bt == 0 and j == 0),
                        stop=(bt == n_big - 1 and j == 3),
                        skip_group_check=True,
                        tile_position=(0, 32 * g),
                    )

    # ---------------- Phase B: s -> r -> xa -> gate ---------------------------
    # s_acc rows 32g+k hold quad-g partial sums (k<21); rows k>=21 are junk
    # from the zero-padded qk columns (exp(0)=1) and are masked by sel4n.
    sg_sb = consts.tile([P, 1], F32, tag="sg_sb")
    nc.vector.reduce_sum(sg_sb[:], s_acc[:, :], axis=mybir.AxisListType.X)
    s_ps = ps_pt.tile([K, 1], F32, tag="pt")
    nc.tensor.matmul(s_ps[:], sel4n_sb[:], sg_sb[:], start=True, stop=True,
                     skip_group_check=True)
    s_sb = consts.tile([K, 1], F32, tag="s_sb")
    nc.vector.tensor_copy(s_sb[:], s_ps[:])
    r_sb = consts.tile([K, 1], F32, tag="r_sb")
    nc.vector.reciprocal(r_sb[:], s_sb[:])

    # y lives in 4 col-group blocks at partitions 32g..32g+20. Replicate r into
    # the same blocks (zeros elsewhere); xa matmul contracts all 128 partitions.
    yf_sb = consts.tile([P, C], F32, tag="yf_sb")
    nc.vector.memset(yf_sb[:], 0.0)
    for g in range(4):
        nc.vector.tensor_copy(yf_sb[32 * g : 32 * g + K, :], y_ps[32 * g : 32 * g + K, :])
    r4_ps = ps_pt.tile([P, 1], F32, tag="pt")
    nc.tensor.matmul(r4_ps[:], selt_sb[:], r_sb[:], start=True, stop=True,
                     skip_group_check=True)
    r4_sb = consts.tile([P, 1], F32, tag="r4_sb")
    nc.vector.tensor_copy(r4_sb[:], r4_ps[:])

    xa_ps = ps_l.tile([1, C], F32, tag="l")
    nc.tensor.matmul(xa_ps[:], r4_sb[:], yf_sb[:], start=True, stop=True)
    xa_sb = consts.tile([1, C], F32, tag="xa_sb")
    nc.vector.tensor_copy(xa_sb[:], xa_ps[:])

    xat_ps = ps_pt.tile([P, 2], F32, tag="pt")
    for j in range(2):
        nc.tensor.transpose(
            xat_ps[:, j : j + 1], xa_sb[0:1, j * P : (j + 1) * P], idf_sb[0:1, 0:1]
        )
    xat_sb = consts.tile([P, 2], F32, tag="xat_sb")
    nc.vector.tensor_copy(xat_sb[:], xat_ps[:])

    gate_ps = ps_l.tile([P, 2], F32, tag="l")
    for cc in range(2):
        csl = slice(cc * P, (cc + 1) * P)
        nc.tensor.matmul(
            gate_ps[:, cc : cc + 1], m2t0[:, csl], xat_sb[:, 0:1],
            start=True, stop=False, skip_group_check=True,
        )
        nc.tensor.matmul(
            gate_ps[:, cc : cc + 1], m2t1[:, csl], xat_sb[:, 1:2],
            start=False, stop=True, skip_group_check=True,
        )
    gate_sb = consts.tile([P, 2], F32, tag="gate_sb")
    nc.vector.tensor_add(gate_sb[:], gate_ps[:], g0_sb[:])

    # ---------------- Phase C: out = x * gate (x already in SBUF) -------------
    mi = 0
    for cc in range(2):
        csl = slice(cc * P, (cc + 1) * P)
        for nt in range(HW // NB):
            xc = xres[cc, nt]
            nsl = slice(nt * NB, (nt + 1) * NB)
            if mi % 2 == 0:
                nc.vector.tensor_scalar_mul(xc[:], xc[:], gate_sb[:, cc : cc + 1])
            else:
                nc.scalar.mul(xc[:], xc[:], gate_sb[:, cc : cc + 1])
            mi += 1
            nc.sync.dma_start(out[csl, nsl], xc[:])


def build_nc(repeats=1, body=None):
    body = body or _body
    nc = bacc.Bacc(
        "TRN2",
        target_bir_lowering=False,
        debug=False,
        enable_asserts=False,
        num_devices=B,
    )
    x = nc.dram_tensor("x", [C, HW], F32, kind="ExternalInput").ap()
    qk16 = nc.dram_tensor("qk16", [C, KP], BF16, kind="ExternalInput").ap()
    m2 = nc.dram_tensor("m2t", [C, C], F32, kind="ExternalInput").ap()
    g0 = nc.dram_tensor("g0", [P, 2], F32, kind="ExternalInput").ap()
    identf = nc.dram_tensor("identf", [P, P], F32, kind="ExternalInput").ap()
    ident16 = nc.dram_tensor("ident16", [P, P], BF16, kind="ExternalInput").ap()
    sel4 = nc.dram_tensor("sel4", [K, P], F32, kind="ExternalInput").ap()
    sel4n = nc.dram_tensor("sel4n", [P, K], F32, kind="ExternalInput").ap()
    out = nc.dram_tensor("out", [C, HW], F32, kind="ExternalOutput").ap()

    with tile.TileContext(nc) as tc:
        for r in range(repeats):
            with ExitStack() as ctx:
                body(ctx, tc, x, qk16, m2, g0, identf, ident16, sel4, sel4n, out,
                     sfx=f"_{r}")
    nc.compile()
    return nc


_NC = None


def _get_nc():
    global _NC
    if _NC is None:
        _NC = build_nc()
    return _NC


def make_in_maps(x, query_embed, Wk, bk, Wv, bv, Wo, bo):
    import ml_dtypes

    x = np.asarray(x, dtype=np.float32)
    qe = np.asarray(query_embed, dtype=np.float64)
    Wk64 = np.asarray(Wk, dtype=np.float64)
    Wv64 = np.asarray(Wv, dtype=np.float64)
    Wo64 = np.asarray(Wo, dtype=np.float64)
    bv64 = np.asarray(bv, dtype=np.float64)
    bo64 = np.asarray(bo, dtype=np.float64)

    qkT = ((qe @ Wk64) / np.sqrt(float(D))).T  # [C, K]
    qk16 = np.zeros((C, KP), dtype=ml_dtypes.bfloat16)
    qk16[:, :K] = qkT.astype(ml_dtypes.bfloat16)
    m2t = (Wo64 @ Wv64).T.astype(np.float32).copy()
    g0 = (float(K) * (Wo64 @ bv64 + bo64)).astype(np.float32)
    g0c = np.ascontiguousarray(g0.reshape(2, P).T)
    identf = np.eye(P, dtype=np.float32)
    ident16 = np.eye(P, dtype=ml_dtypes.bfloat16)
    sel4n = np.zeros((P, K), dtype=np.float32)
    for g in range(4):
        for k in range(K):
            sel4n[32 * g + k, k] = 1.0
    sel4 = np.ascontiguousarray(sel4n.T)

    return [
        {
            "x": np.ascontiguousarray(x[b].reshape(C, HW)),
            "qk16": qk16,
            "m2t": m2t,
            "g0": g0c,
            "identf": identf,
            "ident16": ident16,
            "sel4": sel4,
            "sel4n": sel4n,
        }
        for b in range(B)
    ]


def kernel(x, query_embed, Wk, bk, Wv, bv, Wo, bo, _trace=False, **kw):
    in_maps = make_in_maps(x, query_embed, Wk, bk, Wv, bv, Wo, bo)
    nc = _get_nc()
    res = run_bass_kernel_spmd(nc, in_maps, core_ids=list(range(B)), trace=_trace, **kw)
    out = np.stack(
        [res.results[b]["out"].reshape(C, 128, 128) for b in range(B)]
    ).astype(np.float32)
    if _trace:
        kernel.last_results = res
    return out
